# revision 75
# baseline (speedup 1.0000x reference)
"""Causal multi-head attention block (QKV proj + causal softmax attention + out proj)
for Trainium2, sharded over 8 NeuronCores.

Sharding: tensor-parallel over heads x data-parallel over batch.
  core (b, g) for b in {0,1}, g in {0..3}: batch b, head group g (4 heads of 16).
  Each core computes its 4 heads' attention output slice and a partial
  output projection (row-parallel W_O); host sums the 4 partials per batch.

All on-chip attention math runs in bf16 operands with fp32 PSUM accumulation
(1 cycle/row on the PE at any moving size):
  - Q^T,K^T = (W^T)^T @ x^T on the PE, stored bf16; V stored per k-tile as
    [t,d] bf16 with an appended ones column (softmax denominator).
  - S^T[k,q] = (K^T)^T @ Q^T per 128-wide k-tile over the causally valid
    q-span; each 2-bank PSUM slab holds 1024 score columns (2 k-tiles at
    q-width 512, 4 k-tiles at q-width 256) so ScalarE exps them in a single
    instruction. Causal raggedness of the diagonal tiles is zeroed post-exp
    with a gpsimd affine_select on the bf16 P tile.
  - O[q,d] (+ denominator column) = P^T.T @ V_aug: the P tile is the
    *stationary* operand, so the output has a full 128 q-partitions and only
    65 moving columns.
  - normalize: per-partition reciprocal of the denominator column + scale
    (or a single gpsimd divide in the endgame).
  - attn [q,d] -> attnT [d,q] via PE transposes (128x128, bf16), then
    partial_out[t,o] = attnT.T @ W_O^T.

Scheduling: ScalarE's exp paces the attention inner loop; the PE is the
global bottleneck (~100us of matmul rows), so the point of everything below
is zero PE idle:
  - warm-up matmuls on a memset tile cover the first-DMA latency and finish
    the PE p-state ramp before real work arrives.
  - the host interleaves W_qk with x(tch0) per 128-row contraction chunk
    (one DMA per chunk: the HWDGE descriptor generator is an exclusive
    ~630ns/DMA device, so supply pieces must match consumption order).
  - the q range is processed in TAPERED segments (512,512,512,256,256): the
    causal triangle puts ~44% of all exp work in the last 512 q columns, so
    narrowing the final segments shrinks the ScalarE runway that trails the
    last PE bulk work, without changing exp instruction count (4 k-tiles
    share a slab at width 256).
  - segs 3/4's k-tiles 0..7 are causally full and depend only on Q(t3) and
    K(t0,t1), so their ST+exp precompute into an SBUF stash during segs 0/1
    where ScalarE would otherwise idle -- roughly halving the exp runway
    that remains inside the final segments.
  - each head's ST/exp/mask slabs stream first; each q-subtile's PV runs as
    one contiguous PSUM accumulation chain (a matmul with start=True resets
    its whole PSUM bank, so accumulation groups never interleave within a
    bank -- chains rotate across two O banks, shared with the transposes).
    The previous head's chains interleave between this head's slabs; PE gaps
    are filled with the next chunk's QKV projection and V tiles, paced
    evenly across each seg by a credit counter.
  - ALL W_O tiles land in seg4: after the stash they are the only
    exp-independent PE work left, and seg4 needs a deep PE reservoir next
    to its exp runway; its filler rate is damped so the STs (and thus the
    last exps) come first, leaving a PE-bound finish.
  - endgame: the last head's chains are emitted in slab-gated step groups
    with the leftover W_O tiles woven between them; normalizes use a DVE
    reciprocal + a ScalarE scale-apply (ScalarE can read PSUM and is idle
    after its last exp, gpsimd cannot touch PSUM), and the final W_O tiles
    stream with split copies/DMA. Partial outputs store as fp16 to halve
    output DMA.
"""

import sys

sys.path.insert(0, "/opt/trn_rl_repo")

import numpy as np
import ml_dtypes

import concourse.bacc as bacc
import concourse.mybir as mybir
import concourse.tile as tile
from concourse import bass_utils

B, T, C = 2, 2048, 1024
H, DK = 16, 64
G = 4  # tensor-parallel head groups
HG = H // G  # heads per core
WQ = HG * DK  # 256 Q (=K=V) cols per core
N_CORES = 8
F32 = mybir.dt.float32
F16 = mybir.dt.float16
BF = mybir.dt.bfloat16

CK = C // 128  # 8 contraction chunks
NT = T // 128  # 16 k/t-tiles
QCH = 512  # q chunk for QKV projection
N_WARM = 12  # warm-up matmuls (256 rows each)
SEGS = [(0, 512), (512, 512), (1024, 512), (1536, 256), (1792, 256)]
MM_LABELS = []  # emission-order matmul labels (debug/trace attribution)
_CUR = ["?"]


def _emit(nc, wx, xr, wv, woT, ident, out):
    with tile.TileContext(nc) as tc:
        with (
            tc.tile_pool(name="persist", bufs=1) as persist,
            tc.tile_pool(name="pt", bufs=13) as pt_pool,
            tc.tile_pool(name="small", bufs=6) as small_pool,
            tc.tile_pool(name="ob", bufs=4) as ob_pool,
            tc.tile_pool(name="st_ps", bufs=2, space="PSUM") as st_ps,
            tc.tile_pool(name="o_ps", bufs=2, space="PSUM") as o_ps,
            tc.tile_pool(name="qkv_ps", bufs=2, space="PSUM") as qkv_ps,
        ):
            # wx_all[:, k, 0:512]   = W_qk chunk k (Q j0,j1 | K j2,j3)
            # wx_all[:, k, 512:1024] = x^T chunk k, t in [0, 512)
            wx_all = persist.tile([128, CK, 1024], BF, tag="wx_all")
            xr_all = persist.tile([128, CK, 3 * QCH], BF, tag="xr_all")
            wv_all = persist.tile([128, CK, WQ], BF, tag="wv_all")
            woT_all = persist.tile([128, 2, C], BF, tag="woT_all")
            qkT = persist.tile([128, 4, T], BF, tag="qkT")
            vaug = persist.tile([128, NT, HG, DK + 1], BF, tag="vaug")
            attnT = persist.tile([128, 2, T], BF, tag="attnT")
            stage = persist.tile([128, 2, 4, 128], BF, tag="stage")
            ident_sb = persist.tile([128, 128], BF, tag="ident_sb")
            warm = persist.tile([128, 256], BF, tag="warm")
            # exp stash: segs 3/4's k-tiles 0..7 are causally full and only
            # need Q(t3) + K(t0,t1), so their ST+exp precompute during the
            # early segs where ScalarE otherwise idles (it holds ~44% of the
            # triangle's exp area otherwise trailing at the end).
            pstash = persist.tile([128, 16, 2, 512], BF, tag="pstash")

            def mm(*a, **k):
                MM_LABELS.append(_CUR[0])
                return nc.tensor.matmul(*a, **k)

            def trp(*a, **k):
                MM_LABELS.append(_CUR[0])
                return nc.tensor.transpose(*a, **k)

            # Warm-up: keep the PE busy through the first-DMA latency window
            # and complete the p-state ramp so real work runs at full clock.
            nc.gpsimd.memset(warm[:], 0.0)
            _CUR[0] = "warm"
            wps = qkv_ps.tile([128, 256], F32, tag="mm", name="warm_ps")
            for _ in range(N_WARM):
                mm(wps[:], warm[:, 0:128], warm[:], start=True, stop=True)
            # dummy reader: the BIR verifier rejects never-read PSUM writes
            nc.vector.tensor_copy(warm[:, 0:4], wps[:, 0:4])

            # Input DMAs, all on the SP queue in consumption order. The wx
            # pieces are one-DMA-per-contraction-chunk so the QKV chunk0
            # k-outer loop can start on chunk k as soon as its piece lands.
            for k in range(CK):
                nc.sync.dma_start(
                    wx_all[:, k : k + 1, :], wx[k * 128 : (k + 1) * 128, :]
                )
            nc.sync.dma_start(
                wv_all[:], wv[:].rearrange("(k p) c -> p k c", p=128)
            )
            nc.sync.dma_start(
                xr_all[:, :, 0:QCH],
                xr[:, 0:QCH].rearrange("(k p) c -> p k c", p=128),
            )
            nc.sync.dma_start(ident_sb[:], ident[:])
            for tch in (1, 2):
                nc.sync.dma_start(
                    xr_all[:, :, tch * QCH : (tch + 1) * QCH],
                    xr[:, tch * QCH : (tch + 1) * QCH].rearrange(
                        "(k p) c -> p k c", p=128
                    ),
                )
            nc.sync.dma_start(
                woT_all[:], woT[:].rearrange("(j p) c -> p j c", p=128)
            )
            nc.gpsimd.memset(vaug[:, :, :, DK : DK + 1], 1.0)

            def x_tile(k, ti):
                # x^T chunk k, t-tile ti: tiles 0-3 ride the fused wx pieces
                if ti < 4:
                    return wx_all[:, k, QCH + ti * 128 : QCH + (ti + 1) * 128]
                return xr_all[:, k, (ti - 4) * 128 : (ti - 3) * 128]

            def x_span(k, tch):
                # x^T chunk k, full 512-wide t chunk
                if tch == 0:
                    return wx_all[:, k, QCH : 2 * QCH]
                return xr_all[:, k, (tch - 1) * QCH : tch * QCH]

            def qk_mm(ps, j, tch, k):
                _CUR[0] = f"qk j{j} t{tch}"
                mm(
                    ps[:],
                    wx_all[:, k, j * 128 : (j + 1) * 128],
                    x_span(k, tch),
                    start=(k == 0),
                    stop=(k == CK - 1),
                )

            def emit_qk(j, tch):
                ps = qkv_ps.tile([128, QCH], F32, tag="mm", name="ps_qk")
                for k in range(CK):
                    qk_mm(ps, j, tch, k)
                nc.vector.tensor_copy(qkT[:, j, tch * QCH : (tch + 1) * QCH], ps[:])

            def emit_v(ti):
                ps = qkv_ps.tile([128, WQ], F32, tag="mm", name="ps_v")
                _CUR[0] = f"v {ti}"
                for k in range(CK):
                    mm(
                        ps[:],
                        x_tile(k, ti),
                        wv_all[:, k, :],
                        start=(k == 0),
                        stop=(k == CK - 1),
                    )
                nc.vector.tensor_copy(
                    vaug[:, ti, :, 0:DK],
                    ps[:].rearrange("p (h d) -> p h d", h=HG),
                )

            def emit_qkv_chunk0():
                # k-outer over all four Q/K tiles so the PE consumes each
                # arriving wx piece immediately; copies split DVE/Act.
                ps_j = {
                    j: qkv_ps.tile([128, QCH], F32, tag="mm", name=f"ps_j{j}")
                    for j in (0, 1)
                }
                ps_j[2] = st_ps.tile([128, 2, QCH], F32, tag="st", name="ps_j2")
                ps_j[3] = st_ps.tile([128, 2, QCH], F32, tag="st", name="ps_j3")
                for k in range(CK):
                    for j in (0, 2, 1, 3):
                        ps = ps_j[j]
                        dst = ps[:, 0, :] if j >= 2 else ps[:]
                        _CUR[0] = f"qk j{j} t0"
                        mm(
                            dst,
                            wx_all[:, k, j * 128 : (j + 1) * 128],
                            x_span(k, 0),
                            start=(k == 0),
                            stop=(k == CK - 1),
                        )
                # j0/j2 gate the first ST slab: split their copies across
                # Act+DVE halves so both land ~250ns sooner
                for j in (0, 2):
                    src = ps_j[j][:, 0, :] if j >= 2 else ps_j[j][:]
                    nc.scalar.copy(qkT[:, j, 0:256], src[:, 0:256])
                    nc.vector.tensor_copy(qkT[:, j, 256:QCH], src[:, 256:QCH])
                for j in (1, 3):
                    src = ps_j[j][:, 0, :] if j >= 2 else ps_j[j][:]
                    nc.scalar.copy(qkT[:, j, 0:256], src[:, 0:256])
                    nc.vector.tensor_copy(qkT[:, j, 256:QCH], src[:, 256:QCH])

            def emit_wo(ti, scalar_copy=False, split_dma=False, last=False):
                ob = ob_pool.tile([128, 2, QCH], F16, tag="ob")
                for oc in range(2):
                    ps = qkv_ps.tile([128, QCH], F32, tag="mm", name="ps_wo")
                    _CUR[0] = f"wo {ti}"
                    for j in range(2):
                        mm(
                            ps[:],
                            attnT[:, j, ti * 128 : (ti + 1) * 128],
                            woT_all[:, j, oc * QCH : (oc + 1) * QCH],
                            start=(j == 0),
                            stop=(j == 1),
                        )
                    if last and oc == 1:
                        # final store: copy halves on ScalarE+DVE in
                        # parallel so the last DMA issues ~350ns sooner
                        nc.scalar.copy(ob[:, oc, 0:256], ps[:, 0:256])
                        nc.vector.tensor_copy(ob[:, oc, 256:QCH], ps[:, 256:QCH])
                    elif scalar_copy and oc == 0:
                        nc.scalar.copy(ob[:, oc, :], ps[:])
                    else:
                        nc.vector.tensor_copy(ob[:, oc, :], ps[:])
                    if split_dma:
                        # start each half's store as soon as its copy lands
                        nc.sync.dma_start(
                            out[
                                ti * 128 : (ti + 1) * 128,
                                oc * QCH : (oc + 1) * QCH,
                            ],
                            ob[:, oc, :],
                        )
                if not split_dma:
                    nc.sync.dma_start(
                        out[ti * 128 : (ti + 1) * 128, :],
                        ob[:].rearrange("p a b -> p (a b)"),
                    )

            def emit_tr1(si, hp, qt, scalar_copy=False):
                # single 128x128 PE transpose; each gets its own PSUM slot in
                # the o ring (a second transpose into the same bank would
                # clear the first: matmul start resets the whole bank)
                q0, qw = SEGS[si]
                tr = o_ps.tile([128, 128], BF, tag="o", name="tr1")
                _CUR[0] = f"tr s{si} hp{hp} q{qt}"
                trp(tr[:], stage[:, hp, qt, :], ident_sb[:])
                cp = nc.scalar.copy if scalar_copy else nc.vector.tensor_copy
                cp(
                    attnT[:, hp, q0 + qt * 128 : q0 + (qt + 1) * 128],
                    tr[:],
                )

            def emit_pre(si, h, sl):
                # precompute ST+exp for seg si's k-tiles [4sl, 4sl+4) into
                # the stash (no diagonal here, so no mask needed)
                q0, qw = SEGS[si]
                prow = (h % 2) * 64
                QT_h = qkT[prow : prow + 64, h // 2, :]
                KT_h = qkT[prow : prow + 64, 2 + h // 2, :]
                idxs = ((si - 3) * HG + h) * 2 + sl
                st = st_ps.tile([128, 2, 512], F32, tag="st", name="st")
                for idx in range(4):
                    kk = sl * 4 + idx
                    off = (idx % 2) * qw
                    _CUR[0] = f"pst s{si} h{h} sl{sl}"
                    mm(
                        st[:, idx // 2, off : off + qw],
                        KT_h[:, 128 * kk : 128 * (kk + 1)],
                        QT_h[:, q0 : q0 + qw],
                        start=True,
                        stop=True,
                    )
                nc.scalar.activation(
                    pstash[:, idxs],
                    st[:, :, :],
                    mybir.ActivationFunctionType.Exp,
                    scale=float(1.0 / np.sqrt(DK)),
                )

            # filler machinery: one slab-slot at a time between attention
            # work, paced evenly across the seg's slots by a credit counter
            fillers = []
            pace = {"credit": 0.0, "rate": 0.0}

            def drain():
                pace["credit"] += pace["rate"]
                while fillers and pace["credit"] >= 1.0:
                    fillers.pop(0)()
                    pace["credit"] -= 1.0

            # Attention. A slab is a 2-bank PSUM tile [128, 2, 512] viewed
            # as SLK = 1024 // qw score tiles of q-width qw; slab sl covers
            # k-tiles [sl*SLK, min((sl+1)*SLK, nk)).
            pieces = []

            def pv_steps(o_t, si, h, qt, pts, a, b, nsteps):
                q0, qw = SEGS[si]
                SLK = 1024 // qw
                sub = 512 // qw
                _CUR[0] = f"pv s{si} h{h} q{qt}"
                for kk in range(a, b):
                    idx = kk % SLK
                    off = (idx % sub) * qw if sub > 1 else 0
                    mm(
                        o_t[:],
                        pts[kk // SLK][
                            :, idx // sub, off + 128 * qt : off + 128 * (qt + 1)
                        ],
                        vaug[:, kk, h, :],
                        start=(kk == 0),
                        stop=(kk == nsteps - 1),
                    )

            def pv_norm(o_t, si, h, qt, eng):
                dst = stage[:, h // 2, qt, (h % 2) * DK : (h % 2 + 1) * DK]
                rc = small_pool.tile([128, 1], F32, tag="rc", name="rc")
                if eng == "act":
                    # endgame path: the tiny reciprocal rides DVE but the
                    # PSUM-heavy scale-apply goes to ScalarE (PSUM-capable,
                    # idle after its last exp), dodging DVE's copy queue.
                    nc.vector.reciprocal(rc[:], o_t[:, DK : DK + 1])
                    nc.scalar.activation(
                        dst, o_t[:, 0:DK],
                        mybir.ActivationFunctionType.Copy, scale=rc[:],
                    )
                else:
                    nc.vector.reciprocal(rc[:], o_t[:, DK : DK + 1])
                    nc.vector.tensor_scalar(
                        dst, o_t[:, 0:DK], rc[:], None, mybir.AluOpType.mult,
                    )

            def pv_chain(si, h, qt, pts, eng=None):
                q0, _ = SEGS[si]
                nsteps = q0 // 128 + qt + 1
                o_t = o_ps.tile([128, DK + 1], F32, tag="o", name="o_t")
                pv_steps(o_t, si, h, qt, pts, 0, nsteps, nsteps)
                pv_norm(o_t, si, h, qt, eng)

            def pv_piece(si, h, qt, pts):
                def go():
                    pv_chain(si, h, qt, pts)
                    if h % 2 == 1 and qt % 2 == 1:
                        emit_tr1(si, h // 2, qt - 1)
                        emit_tr1(si, h // 2, qt)
                return go

            def pv_piece_split(si, h, qt, pts):
                # stashed segs: steps 0..11 are gated on long-done stash
                # exps, the tail on this head's final slab exp -- splitting
                # lets the tail drain one slab later with real slack
                q0, _ = SEGS[si]
                nsteps = q0 // 128 + qt + 1
                cell = {}

                def goa():
                    cell["o"] = o_ps.tile(
                        [128, DK + 1], F32, tag="o", name="o_t"
                    )
                    pv_steps(cell["o"], si, h, qt, pts, 0, 12, nsteps)

                def gob():
                    pv_steps(cell["o"], si, h, qt, pts, 12, nsteps, nsteps)
                    pv_norm(cell["o"], si, h, qt, None)
                    if h % 2 == 1 and qt % 2 == 1:
                        emit_tr1(si, h // 2, qt - 1)
                        emit_tr1(si, h // 2, qt)
                return goa, gob

            def emit_head(si, h, last=False):
                q0, qw = SEGS[si]
                SLK = 1024 // qw
                sub = 512 // qw
                nk = (q0 + qw) // 128
                nsl = (nk + SLK - 1) // SLK
                nqt = qw // 128
                prow = (h % 2) * 64
                QT_h = qkT[prow : prow + 64, h // 2, :]
                KT_h = qkT[prow : prow + 64, 2 + h // 2, :]
                pts = []
                if si >= 3:
                    # slabs 0/1 were precomputed into the stash
                    for psl in (0, 1):
                        pts.append(pstash[:, ((si - 3) * HG + h) * 2 + psl])
                for sl in range(len(pts), nsl):
                    ntiles = min(SLK, nk - sl * SLK)
                    st = st_ps.tile([128, 2, 512], F32, tag="st", name="st")
                    pt = pt_pool.tile([128, 2, 512], BF, tag="pt", name="pt")
                    for idx in range(ntiles):
                        kk = sl * SLK + idx
                        d0 = max(0, 128 * kk - q0)
                        off = (idx % sub) * qw if sub > 1 else 0
                        _CUR[0] = f"st s{si} h{h} sl{sl}"
                        mm(
                            st[:, idx // sub, off + d0 : off + qw],
                            KT_h[:, 128 * kk : 128 * (kk + 1)],
                            QT_h[:, q0 + d0 : q0 + qw],
                            start=True,
                            stop=True,
                        )
                    # exp the slab in one instruction; for q-width 512 the
                    # leading diagonal trim (d0 of the first tile) is cut,
                    # narrower segs exp the full slab (the ragged area is
                    # never read by the chains).
                    nj = (ntiles + sub - 1) // sub
                    d0m = max(0, 128 * sl * SLK - q0) if sub == 1 else 0
                    nc.scalar.activation(
                        pt[:, 0:nj, d0m:512],
                        st[:, 0:nj, d0m:512],
                        mybir.ActivationFunctionType.Exp,
                        scale=float(1.0 / np.sqrt(DK)),
                    )
                    for idx in range(ntiles):
                        kk = sl * SLK + idx
                        d0 = max(0, 128 * kk - q0)
                        if 128 * kk >= q0:
                            off = (idx % sub) * qw if sub > 1 else 0
                            # zero p where q < k inside the ragged diag block
                            nc.gpsimd.affine_select(
                                out=pt[:, idx // sub, off + d0 : off + d0 + 128],
                                in_=pt[:, idx // sub, off + d0 : off + d0 + 128],
                                compare_op=mybir.AluOpType.is_ge,
                                fill=0.0,
                                base=0,
                                pattern=[[1, 128]],
                                channel_multiplier=-1,
                            )
                    pts.append(pt)
                    if pieces and (sl >= 1 or nsl <= 2):
                        n = (len(pieces) + (nsl - sl) - 1) // (nsl - sl)
                        for p in pieces[:n]:
                            p()
                        del pieces[:n]
                    drain()
                if last:
                    # Endgame weave (final 256-wide seg: chains of 15 and 16
                    # steps; steps 12+ are gated on the final slab's exp).
                    # The reserved W_O tiles are woven between slab-gated
                    # step groups so the PE absorbs ScalarE's remaining exp
                    # latency instead of stalling inside a chain. Normalizes
                    # ride ScalarE (PSUM-capable, idle after its last exp)
                    # so the transposes don't queue behind DVE's W_O copies.
                    n0 = q0 // 128 + 1  # 15
                    n1 = q0 // 128 + 2  # 16
                    q0t = o_ps.tile([128, DK + 1], F32, tag="o", name="o_t")
                    pv_steps(q0t, si, h, 0, pts, 0, 8, n0)
                    q1t = o_ps.tile([128, DK + 1], F32, tag="o", name="o_t")
                    pv_steps(q1t, si, h, 1, pts, 0, 8, n1)
                    half = (len(fillers) + 1) // 2
                    for f in fillers[:half]:  # absorb slab-2 exp lag
                        f()
                    pv_steps(q0t, si, h, 0, pts, 8, 12, n0)
                    pv_steps(q1t, si, h, 1, pts, 8, 12, n1)
                    for f in fillers[half:]:  # absorb final-slab exp lag
                        f()
                    del fillers[:]
                    emit_wo(12, scalar_copy=True)
                    emit_wo(13, scalar_copy=True)
                    pv_steps(q0t, si, h, 0, pts, 12, n0, n0)
                    pv_norm(q0t, si, h, 0, "act")
                    emit_tr1(si, h // 2, 0)
                    pv_steps(q1t, si, h, 1, pts, 12, n1, n1)
                    pv_norm(q1t, si, h, 1, "act")
                    emit_tr1(si, h // 2, 1, scalar_copy=True)
                    emit_wo(14, scalar_copy=True)
                    emit_wo(15, scalar_copy=True, split_dma=True)
                elif si >= 3:
                    subs = [pv_piece_split(si, h, qt, pts) for qt in range(nqt)]
                    for s in subs:
                        pieces.append(s[0])
                    for s in subs:
                        pieces.append(s[1])
                else:
                    for qt in range(nqt):
                        pieces.append(pv_piece(si, h, qt, pts))

            emit_qkv_chunk0()
            NSEG = len(SEGS)
            for si in range(NSEG):
                q0, qw = SEGS[si]
                SLK = 1024 // qw
                nk = (q0 + qw) // 128
                nsl = (nk + SLK - 1) // SLK
                del fillers[:]
                if si == 0:
                    # V(0..3) rides the first slabs (PV lags the slab
                    # stream, so early vaug tiles arrive in time); K(t1) and
                    # Q(t3) unlock the seg3 stash slabs
                    for ti in range(4):
                        fillers.append(lambda ti=ti: emit_v(ti))
                    for j in (2, 3):
                        fillers.append(lambda j=j: emit_qk(j, 1))
                    for j in (0, 1):
                        fillers.append(lambda j=j: emit_qk(j, 3))
                    for hh in range(HG):
                        fillers.append(lambda hh=hh: emit_pre(3, hh, 0))
                    for j in (0, 1):
                        fillers.append(lambda j=j: emit_qk(j, 1))
                    for ti in range(4, 6):
                        fillers.append(lambda ti=ti: emit_v(ti))
                if si == 1:
                    for ti in range(6, 8):
                        fillers.append(lambda ti=ti: emit_v(ti))
                    for hh in range(HG):
                        fillers.append(lambda hh=hh: emit_pre(3, hh, 1))
                    for j in (2, 3, 0, 1):
                        fillers.append(lambda j=j: emit_qk(j, 2))
                    for hh in range(HG):
                        fillers.append(lambda hh=hh: emit_pre(4, hh, 0))
                if si == 2:
                    for ti in range(8, 12):
                        fillers.append(lambda ti=ti: emit_v(ti))
                    for j in (2, 3):
                        fillers.append(lambda j=j: emit_qk(j, 3))
                    for hh in range(HG):
                        fillers.append(lambda hh=hh: emit_pre(4, hh, 1))
                if si == 3:
                    for ti in range(12, 14):
                        fillers.append(lambda ti=ti: emit_v(ti))
                    for ti in range(14, 16):
                        fillers.append(lambda ti=ti: emit_v(ti))
                if si == 4:
                    # ALL W_O lands in seg4: it is the only exp-independent
                    # PE work left, and seg4's exp runway (~17us) needs an
                    # equally deep PE reservoir beside it. 12..15 are
                    # reserved for the endgame weave.
                    for ti in range(0, 12):
                        fillers.append(lambda ti=ti: emit_wo(ti))
                in_sl = nsl - (2 if si >= 3 else 0)
                pace["rate"] = len(fillers) / float(HG * in_sl)
                if si == NSEG - 1:
                    # the stash leaves seg4 ACT-light: prioritize its STs so
                    # the last exp lands early, and leave most of the W_O
                    # backlog for the endgame weave (PE-bound finish)
                    pace["rate"] /= 3.0
                pace["credit"] = 1.0 if si < NSEG - 1 else pace["rate"]
                for h in range(HG):
                    emit_head(si, h, last=(si == NSEG - 1 and h == HG - 1))
                # safety: leftover fillers at seg end
                flushed = list(fillers)
                del fillers[:]
                for f in flushed:
                    f()
            for p in pieces:
                p()
            del pieces[:]


_CACHE = {}


def _build():
    if "nc" in _CACHE:
        return _CACHE["nc"]
    nc = bacc.Bacc("TRN2", debug=False, num_devices=N_CORES)
    wx = nc.dram_tensor("wx", [C, 1024], BF, kind="ExternalInput").ap()
    xr = nc.dram_tensor("xr", [C, 3 * QCH], BF, kind="ExternalInput").ap()
    wv = nc.dram_tensor("wv", [C, WQ], BF, kind="ExternalInput").ap()
    woT = nc.dram_tensor("woT", [2 * 128, C], BF, kind="ExternalInput").ap()
    ident = nc.dram_tensor("ident", [128, 128], BF, kind="ExternalInput").ap()
    out = nc.dram_tensor("out", [T, C], F16, kind="ExternalOutput").ap()
    _emit(nc, wx, xr, wv, woT, ident, out)
    nc.compile()
    _CACHE["nc"] = nc
    return nc


_IDENT = np.eye(128, dtype=ml_dtypes.bfloat16)


def _shard_inputs(x, W_QKV, W_O):
    """Build the 8 per-core input maps. core = b*G + g."""
    in_maps = []
    W_Q, W_K, W_V = W_QKV[0:C], W_QKV[C : 2 * C], W_QKV[2 * C : 3 * C]
    xT_b = [
        np.ascontiguousarray(x[b].T).astype(ml_dtypes.bfloat16) for b in range(B)
    ]
    for b in range(B):
        xT = xT_b[b]
        for g in range(G):
            sl = slice(g * HG * DK, (g + 1) * HG * DK)
            wqkT_g = np.concatenate([W_Q[sl], W_K[sl]], axis=0).T  # [C, 512]
            wx_g = np.ascontiguousarray(
                np.concatenate([wqkT_g, xT[:, 0:QCH]], axis=1)
            ).astype(ml_dtypes.bfloat16)
            wv_g = np.ascontiguousarray(W_V[sl].T).astype(ml_dtypes.bfloat16)
            xr_g = np.ascontiguousarray(xT[:, QCH:])
            woT_g = np.ascontiguousarray(W_O[:, sl].T).astype(ml_dtypes.bfloat16)
            in_maps.append(
                {
                    "wx": wx_g,
                    "xr": xr_g,
                    "wv": wv_g,
                    "woT": woT_g,
                    "ident": _IDENT,
                }
            )
    return in_maps


def kernel(x, W_QKV, W_O):
    x = np.asarray(x, dtype=np.float32)
    W_QKV = np.asarray(W_QKV, dtype=np.float32)
    W_O = np.asarray(W_O, dtype=np.float32)
    nc = _build()
    in_maps = _shard_inputs(x, W_QKV, W_O)
    res = bass_utils.run_bass_kernel_spmd(
        nc, in_maps, core_ids=list(range(N_CORES))
    )
    out = np.zeros((B, T, C), dtype=np.float32)
    for b in range(B):
        for g in range(G):
            out[b] += np.asarray(res.results[b * G + g]["out"], dtype=np.float32)
    return out
